# revision 12
# baseline (speedup 1.0000x reference)
"""CQAttention Trainium2 kernel.

Math (per batch b):
  S = (C*w3) @ Q^T + (C@w1)[:,None] + (Q@w2)[None,:] (+bias, dropped: softmax-invariant)
  Sq = softmax over q of qmask-masked S ; Sc = softmax over c of cmask-masked S
  A = Sq@Q ; Bm = Sq @ (Sc^T @ C) ; out = [C | A | C*A | C*Bm]

Device algorithm (no max-subtraction: |S| < 1 so exp is safe; masks become
additive -1e30 terms):
  ST   = (Q*w3) @ C^T                        [q, c]  (PE; stationary QT3)
  E_q  = exp(ST + (rq + qneg)[q])            [q, c]  row-masked (ACT bias)
  STT  = C @ [(Q*w3)^T | w1 w1]              [c, q+2] (PE; stationary CT tile;
         cols 128:130 accumulate rc = C@w1 for free)
  Xg   = exp(STT[:, :128] + (rc + cneg)[c])  [c, q]  col-mask+rc via ACT bias
  T1s  = (Xg^T @ [C|1]) normalized           [q, d]  == Sc^T @ C
  psA  = E_q^T @ [Q|1]                       [c, d+2] unnormalized A | rowsum
  psB  = E_q^T @ T1s                         [c, d]   unnormalized Bm
  A = psA * rr ; CA = C * A ; CBm = C * psB * rr      (rr = 1/rowsum)

Layout: c-row mapping c = 8*p + j (partition p owns 8 consecutive HBM rows),
so the per-batch output store is one DMA with 32KB contiguous per partition,
and C is DMA'd directly into the output tile (no engine copy, no extra store).

Sharding: data-parallel over batch, 4 batches per core on 8 cores.
"""

import numpy as np

NEG_INF = -1e30
B_FULL, LC, LQ, D = 32, 1024, 128, 256
N_CORES = 8
NB = B_FULL // N_CORES  # batches per core
KC = LC // 128  # c-tiles per batch (8)

_CACHE = {}


def _build_nc():
    import concourse.bacc as bacc
    import concourse.mybir as mybir
    from concourse import tile
    from concourse.masks import make_identity

    fp32 = mybir.dt.float32
    fp32r = mybir.dt.float32r
    mmdt = fp32r
    MULT = mybir.AluOpType.mult
    ADD = mybir.AluOpType.add
    EXP = mybir.ActivationFunctionType.Exp

    nc = bacc.Bacc("TRN2", target_bir_lowering=False, debug=False)

    C_d = nc.dram_tensor("C", [NB, LC, D], fp32, kind="ExternalInput")
    Q_d = nc.dram_tensor("Q", [NB, LQ, D], fp32, kind="ExternalInput")
    cneg_d = nc.dram_tensor("cneg", [NB, 128, KC], fp32, kind="ExternalInput")
    qneg_d = nc.dram_tensor("qneg", [NB, 128, 1], fp32, kind="ExternalInput")
    w_d = nc.dram_tensor("w_pk", [128, 6], fp32, kind="ExternalInput")
    w2bc_d = nc.dram_tensor("w2bc", [128, D], fp32, kind="ExternalInput")
    out_d = nc.dram_tensor("out", [NB, LC, 4 * D], fp32, kind="ExternalOutput")

    with tile.TileContext(nc) as tc:
        with (
            tc.tile_pool(name="const", bufs=1) as const,
            tc.tile_pool(name="crpool", bufs=2) as p_cr,
            tc.tile_pool(name="qpool", bufs=NB) as p_q,
            tc.tile_pool(name="mpool", bufs=NB) as p_m,
            tc.tile_pool(name="ctpool", bufs=2) as p_ct,
            tc.tile_pool(name="qtpool", bufs=2) as p_qt,
            tc.tile_pool(name="epool", bufs=2) as p_e,
            tc.tile_pool(name="xgpool", bufs=2) as p_xg,
            tc.tile_pool(name="opool", bufs=3) as p_o,
            tc.tile_pool(name="smpool", bufs=4) as p_sm,
            tc.tile_pool(name="pst", bufs=1, space="PSUM") as ps_t,
            tc.tile_pool(name="psst", bufs=2, space="PSUM") as ps_st,
            tc.tile_pool(name="psstt", bufs=2, space="PSUM") as ps_stt,
            tc.tile_pool(name="pst1", bufs=1, space="PSUM") as ps_t1,
            tc.tile_pool(name="psa", bufs=1, space="PSUM") as ps_a,
            tc.tile_pool(name="psb", bufs=1, space="PSUM") as ps_b,
        ):
            ident = const.tile([128, 128], fp32)
            make_identity(nc, ident)
            w_sb = const.tile([128, 6], fp32)
            nc.sync.dma_start(w_sb, w_d.ap())
            w2bc = const.tile([128, D], fp32)
            nc.sync.dma_start(w2bc, w2bc_d.ap())
            # duplicated-column w1 (fp32r; rides as 2 extra moving cols of STT)
            w1r2 = const.tile([128, 2, 2], mmdt, tag="w1r2")
            for dk in range(2):
                for j in range(2):
                    nc.vector.tensor_copy(w1r2[:, dk, j : j + 1], w_sb[:, dk : dk + 1])
            # warm up the ACT exp table while input DMAs run
            wtmp = const.tile([128, 1], fp32, tag="wtmp")
            nc.scalar.activation(wtmp, w_sb[:, 0:1], EXP)

            # C1r slots: fp32r copies of C for the T1 matmul moving operand.
            # The two ones-pad columns are written once per slot and persist.
            cr_slots = []
            for _ in range(2):
                cr = p_cr.tile([128, KC, D + 2], mmdt, tag="cr")
                nc.vector.memset(cr.bitcast(fp32)[:, :, D : D + 2], 1.0)
                cr_slots.append(cr)

            # ---- hoisted input loads for all batches ----
            osbs, Q1s, cnegs, qnegs = [], [], [], []
            for b in range(NB):
                # output accumulation tile [C | A | C*A | C*Bm]; C lands in
                # cols 0:D straight from HBM (c = 8p + j row mapping).
                osb = p_o.tile([128, KC, 4 * D], fp32, tag="osb")
                nc.sync.dma_start(
                    osb[:, :, 0:D], C_d.ap()[b].rearrange("(p j) d -> p j d", p=128)
                )
                Q1 = p_q.tile([128, D + 2], fp32, tag="q")
                nc.vector.memset(Q1[:, D : D + 2], 1.0)
                nc.sync.dma_start(Q1[:, 0:D], Q_d.ap()[b])
                cneg = p_m.tile([128, KC], fp32, tag="cneg")
                nc.sync.dma_start(cneg, cneg_d.ap()[b])
                qneg = p_m.tile([128, 1], fp32, tag="qneg")
                nc.sync.dma_start(qneg, qneg_d.ap()[b])
                osbs.append(osb)
                Q1s.append(Q1)
                cnegs.append(cneg)
                qnegs.append(qneg)

            for b in range(NB):
                osb, Q1, cneg, qneg = osbs[b], Q1s[b], cnegs[b], qnegs[b]
                Cb = osb[:, :, 0:D]

                # rounded fp32r copies for matmul moving operands
                C1r = cr_slots[b % 2]
                nc.vector.tensor_copy(C1r[:, :, 0:D], Cb)
                Q1r = p_q.tile([128, D + 2], mmdt, tag="qr")
                nc.vector.tensor_copy(Q1r, Q1)

                # ---- bias_q = qneg + sum_d Q*w2 ----
                scr = p_sm.tile([128, D], fp32, tag="ttrs")
                nc.gpsimd.tensor_mul(scr, Q1[:, 0:D], w2bc)
                rq = p_sm.tile([128, 1], fp32, tag="rq")
                nc.vector.tensor_reduce(rq, scr, mybir.AxisListType.X, ADD)
                bias_q = p_sm.tile([128, 1], fp32, tag="biasq")
                nc.vector.tensor_add(bias_q, rq, qneg)

                # ---- QT3aug = [(Q^T)*w3 | w1 w1] per d-chunk ----
                QT3 = p_qt.tile([128, 2, 130], mmdt, tag="qtw3")
                pt = ps_t.tile([128, 512], fp32, tag="pt")
                for dk in range(2):
                    nc.tensor.transpose(
                        pt[:, dk * 128 : (dk + 1) * 128],
                        Q1[:, dk * 128 : (dk + 1) * 128],
                        ident,
                    )
                for dk in range(2):
                    nc.vector.tensor_scalar_mul(
                        QT3[:, dk, 0:128],
                        pt[:, dk * 128 : (dk + 1) * 128],
                        w_sb[:, 4 + dk : 5 + dk],
                    )
                    nc.vector.tensor_copy(QT3[:, dk, 128:130], w1r2[:, dk])

                # ---- CT (transpose C): 4 transposes per PSUM bank, 1 copy ----
                CT = p_ct.tile([128, 2, LC], mmdt, tag="ct")
                for dk in range(2):
                    for h in range(2):
                        pt = ps_t.tile([128, 512], fp32, tag="pt")
                        for jj in range(4):
                            j = h * 4 + jj
                            nc.tensor.transpose(
                                pt[:, jj * 128 : (jj + 1) * 128],
                                Cb[:, j, dk * 128 : (dk + 1) * 128],
                                ident,
                            )
                        dst = CT[:, dk, h * 512 : (h + 1) * 512]
                        if (dk * 2 + h) % 2 == 0:
                            nc.scalar.copy(dst, pt)
                        else:
                            nc.vector.tensor_copy(dst, pt)

                # ---- ST = (Q*w3) @ C^T, E_q = exp(ST + bias_q) ----
                E_q = p_e.tile([128, LC], mmdt, tag="eq")
                for h in range(2):
                    st = ps_st.tile([128, 512], fp32, tag="st")
                    for dk in range(2):
                        nc.tensor.matmul(
                            st,
                            QT3[:, dk, 0:128],
                            CT[:, dk, h * 512 : (h + 1) * 512],
                            start=(dk == 0),
                            stop=(dk == 1),
                        )
                    nc.scalar.activation(
                        E_q[:, h * 512 : (h + 1) * 512], st, EXP, bias=bias_q
                    )

                # ---- STT tiles: [c, q | rc] ; Xg = exp(STT + rc + cneg) ----
                Xg = p_xg.tile([128, KC, 128], mmdt, tag="xg")
                for j in range(KC):
                    stt = ps_stt.tile([128, 130], fp32, tag="stt")
                    for dk in range(2):
                        nc.tensor.matmul(
                            stt,
                            CT[:, dk, j * 128 : (j + 1) * 128],
                            QT3[:, dk],
                            start=(dk == 0),
                            stop=(dk == 1),
                        )
                    bias_c = p_sm.tile([128, 1], fp32, tag="biasc")
                    nc.vector.tensor_add(bias_c, stt[:, 128:129], cneg[:, j : j + 1])
                    nc.scalar.activation(Xg[:, j], stt[:, 0:128], EXP, bias=bias_c)

                # ---- T1s = (Sc^T C) = (Xg^T @ [C|1]) normalized ----
                t1 = ps_t1.tile([128, D + 2], fp32, tag="t1")
                for j in range(KC):
                    nc.tensor.matmul(
                        t1,
                        Xg[:, j],
                        C1r[:, j],
                        start=(j == 0),
                        stop=(j == KC - 1),
                    )
                recipT = p_sm.tile([128, 1], fp32, tag="recipT")
                nc.vector.reciprocal(recipT, t1[:, D : D + 1])
                T1s = p_sm.tile([128, D], mmdt, tag="t1s")
                nc.vector.tensor_scalar_mul(T1s, t1[:, 0:D], recipT)

                # ---- per c-tile: A / CA / CBm into osb segments ----
                for j in range(KC):
                    eq_j = E_q[:, j * 128 : (j + 1) * 128]
                    psA = ps_a.tile([128, D + 2], fp32, tag="psa")
                    nc.tensor.matmul(psA, eq_j, Q1r[:], start=True, stop=True)
                    psB = ps_b.tile([128, D], fp32, tag="psb")
                    nc.tensor.matmul(psB, eq_j, T1s[:], start=True, stop=True)

                    rr = p_sm.tile([128, 1], fp32, tag="rr")
                    nc.vector.reciprocal(rr, psA[:, D : D + 1])

                    # A = psA * rr  (ACT, per-partition scale)
                    nc.scalar.mul(osb[:, j, D : 2 * D], psA[:, 0:D], rr)
                    # CA = C * A  (GPSIMD, reads the extracted A)
                    nc.gpsimd.tensor_mul(
                        osb[:, j, 2 * D : 3 * D],
                        osb[:, j, 0:D],
                        osb[:, j, D : 2 * D],
                    )
                    # CBm = (psB * rr) * C  (DVE fused)
                    nc.vector.scalar_tensor_tensor(
                        osb[:, j, 3 * D : 4 * D],
                        psB,
                        rr,
                        osb[:, j, 0:D],
                        MULT,
                        MULT,
                    )

                # one 4MB store per batch: 32KB contiguous per partition
                nc.sync.dma_start(
                    out_d.ap()[b].rearrange("(p j) n -> p j n", p=128), osb
                )

    nc.compile()
    return nc


def _get_nc():
    if "nc" not in _CACHE:
        _CACHE["nc"] = _build_nc()
    return _CACHE["nc"]


def _make_in_maps(C, Q, cmask, qmask, Wo_w):
    C = np.ascontiguousarray(C, dtype=np.float32)
    Q = np.ascontiguousarray(Q, dtype=np.float32)
    cneg = ((1.0 - cmask.astype(np.float32)) * NEG_INF).astype(np.float32)
    qneg = ((1.0 - qmask.astype(np.float32)) * NEG_INF).astype(np.float32)
    # c = 8p + j mapping: row-major [128, KC] — plain reshape
    cneg = np.ascontiguousarray(cneg.reshape(B_FULL, 128, KC))
    qneg = np.ascontiguousarray(qneg.reshape(B_FULL, 128, 1))
    Wo_w = Wo_w.astype(np.float32)
    w_pk = np.ascontiguousarray(Wo_w.reshape(6, 128).T)
    w2bc = np.ascontiguousarray(np.broadcast_to(Wo_w[D : 2 * D], (128, D)))
    in_maps = []
    for i in range(N_CORES):
        sl = slice(i * NB, (i + 1) * NB)
        in_maps.append(
            {
                "C": np.ascontiguousarray(C[sl]),
                "Q": np.ascontiguousarray(Q[sl]),
                "cneg": np.ascontiguousarray(cneg[sl]),
                "qneg": np.ascontiguousarray(qneg[sl]),
                "w_pk": w_pk,
                "w2bc": w2bc,
            }
        )
    return in_maps


def kernel(C, Q, cmask, qmask, Wo_w, Wo_b):
    from concourse.bass_utils import run_bass_kernel_spmd

    nc = _get_nc()
    in_maps = _make_in_maps(C, Q, cmask, qmask, Wo_w)
    res = run_bass_kernel_spmd(nc, in_maps, core_ids=list(range(N_CORES)))
    out = np.concatenate([res.results[i]["out"] for i in range(N_CORES)], axis=0)
    return out


# revision 13
# speedup vs baseline: 1.1486x; 1.1486x over previous
"""CQAttention Trainium2 kernel.

Math (per batch b):
  S = (C*w3) @ Q^T + (C@w1)[:,None] + (Q@w2)[None,:] (+bias, dropped: softmax-invariant)
  Sq = softmax over q of qmask-masked S ; Sc = softmax over c of cmask-masked S
  A = Sq@Q ; Bm = Sq @ (Sc^T @ C) ; out = [C | A | C*A | C*Bm]

Device algorithm (no max-subtraction: |S| < 1 so exp is safe; masks become
additive -1e30 terms):
  ST   = (Q*w3) @ C^T                        [q, c]  (PE; stationary QT3)
  E_q  = exp(ST + (rq + qneg)[q])            [q, c]  row-masked (ACT bias)
  STT  = C @ [(Q*w3)^T | w1 w1]              [c, q+2] (PE; stationary CT tile;
         cols 128:130 accumulate rc = C@w1 for free)
  Xg   = exp(STT[:, :128] + (rc + cneg)[c])  [c, q]  col-mask+rc via ACT bias
  T1s  = (Xg^T @ [C|1]) normalized           [q, d]  == Sc^T @ C
  psA  = E_q^T @ [Q|1]                       [c, d+2] unnormalized A | rowsum
  psB  = E_q^T @ T1s                         [c, d]   unnormalized Bm
  A = psA * rr ; CA = C * A ; CBm = C * psB * rr      (rr = 1/rowsum)

Layout: c-row mapping c = 8*p + j (partition p owns 8 consecutive HBM rows),
so the per-batch output store is one DMA with 32KB contiguous per partition,
and C is DMA'd directly into the output tile (no engine copy, no extra store).

Sharding: data-parallel over batch, 4 batches per core on 8 cores.
"""

import numpy as np

NEG_INF = -1e30
B_FULL, LC, LQ, D = 32, 1024, 128, 256
N_CORES = 8
NB = B_FULL // N_CORES  # batches per core
KC = LC // 128  # c-tiles per batch (8)

_CACHE = {}


def _build_nc():
    import concourse.bacc as bacc
    import concourse.mybir as mybir
    from concourse import tile
    from concourse.masks import make_identity

    fp32 = mybir.dt.float32
    fp32r = mybir.dt.float32r
    mmdt = fp32r
    MULT = mybir.AluOpType.mult
    ADD = mybir.AluOpType.add
    EXP = mybir.ActivationFunctionType.Exp

    nc = bacc.Bacc("TRN2", target_bir_lowering=False, debug=False)

    C_d = nc.dram_tensor("C", [NB, LC, D], fp32, kind="ExternalInput")
    Q_d = nc.dram_tensor("Q", [NB, LQ, D], fp32, kind="ExternalInput")
    cneg_d = nc.dram_tensor("cneg", [NB, 128, KC], fp32, kind="ExternalInput")
    qneg_d = nc.dram_tensor("qneg", [NB, 128, 1], fp32, kind="ExternalInput")
    w_d = nc.dram_tensor("w_pk", [128, 6], fp32, kind="ExternalInput")
    w2bc_d = nc.dram_tensor("w2bc", [128, D], fp32, kind="ExternalInput")
    out_d = nc.dram_tensor("out", [NB, LC, 4 * D], fp32, kind="ExternalOutput")

    with tile.TileContext(nc) as tc:
        with (
            tc.tile_pool(name="const", bufs=1) as const,
            tc.tile_pool(name="crpool", bufs=2) as p_cr,
            tc.tile_pool(name="qpool", bufs=NB) as p_q,
            tc.tile_pool(name="mpool", bufs=NB) as p_m,
            tc.tile_pool(name="ctpool", bufs=3) as p_ct,
            tc.tile_pool(name="qtpool", bufs=2) as p_qt,
            tc.tile_pool(name="epool", bufs=2) as p_e,
            tc.tile_pool(name="xgpool", bufs=2) as p_xg,
            tc.tile_pool(name="opool", bufs=3) as p_o,
            tc.tile_pool(name="smpool", bufs=4) as p_sm,
            tc.tile_pool(name="flex", bufs=3, space="PSUM") as ps_flex,
            tc.tile_pool(name="psstt", bufs=2, space="PSUM") as ps_stt,
            tc.tile_pool(name="pst1", bufs=1, space="PSUM") as ps_t1,
            tc.tile_pool(name="psa", bufs=1, space="PSUM") as ps_a,
            tc.tile_pool(name="psb", bufs=1, space="PSUM") as ps_b,
        ):
            ident = const.tile([128, 128], fp32)
            make_identity(nc, ident)
            w_sb = const.tile([128, 6], fp32)
            nc.sync.dma_start(w_sb, w_d.ap())
            w2bc = const.tile([128, D], fp32)
            nc.sync.dma_start(w2bc, w2bc_d.ap())
            # duplicated-column w1 (fp32r; rides as 2 extra moving cols of STT)
            w1r2 = const.tile([128, 2, 2], mmdt, tag="w1r2")
            for dk in range(2):
                for j in range(2):
                    nc.vector.tensor_copy(w1r2[:, dk, j : j + 1], w_sb[:, dk : dk + 1])
            # warm up the ACT exp table while input DMAs run
            wtmp = const.tile([128, 1], fp32, tag="wtmp")
            nc.scalar.activation(wtmp, w_sb[:, 0:1], EXP)

            # C1r slots: fp32r copies of C for the T1 matmul moving operand.
            # The two ones-pad columns are written once per slot and persist.
            cr_slots = []
            for _ in range(2):
                cr = p_cr.tile([128, KC, D + 2], mmdt, tag="cr")
                nc.vector.memset(cr.bitcast(fp32)[:, :, D : D + 2], 1.0)
                cr_slots.append(cr)

            # ---- hoisted input loads for all batches ----
            osbs, Q1s, cnegs, qnegs = [], [], [], []
            for b in range(NB):
                # output accumulation tile [C | A | C*A | C*Bm]; C lands in
                # cols 0:D straight from HBM (c = 8p + j row mapping).
                osb = p_o.tile([128, KC, 4 * D], fp32, tag="osb")
                nc.sync.dma_start(
                    osb[:, :, 0:D], C_d.ap()[b].rearrange("(p j) d -> p j d", p=128)
                )
                Q1 = p_q.tile([128, D + 2], fp32, tag="q")
                nc.vector.memset(Q1[:, D : D + 2], 1.0)
                nc.sync.dma_start(Q1[:, 0:D], Q_d.ap()[b])
                cneg = p_m.tile([128, KC], fp32, tag="cneg")
                nc.sync.dma_start(cneg, cneg_d.ap()[b])
                qneg = p_m.tile([128, 1], fp32, tag="qneg")
                nc.sync.dma_start(qneg, qneg_d.ap()[b])
                osbs.append(osb)
                Q1s.append(Q1)
                cnegs.append(cneg)
                qnegs.append(qneg)

            for b in range(NB):
                osb, Q1, cneg, qneg = osbs[b], Q1s[b], cnegs[b], qnegs[b]
                Cb = osb[:, :, 0:D]

                # rounded fp32r copies for matmul moving operands
                C1r = cr_slots[b % 2]
                nc.vector.tensor_copy(C1r[:, :, 0:D], Cb)
                Q1r = p_q.tile([128, D + 2], mmdt, tag="qr")
                nc.vector.tensor_copy(Q1r, Q1)

                # ---- bias_q = qneg + sum_d Q*w2 ----
                scr = p_sm.tile([128, D], fp32, tag="ttrs")
                nc.gpsimd.tensor_mul(scr, Q1[:, 0:D], w2bc)
                rq = p_sm.tile([128, 1], fp32, tag="rq")
                nc.vector.tensor_reduce(rq, scr, mybir.AxisListType.X, ADD)
                bias_q = p_sm.tile([128, 1], fp32, tag="biasq")
                nc.vector.tensor_add(bias_q, rq, qneg)

                # ---- QT3aug = [(Q^T)*w3 | w1 w1] per d-chunk ----
                QT3 = p_qt.tile([128, 2, 130], mmdt, tag="qtw3")
                pt = ps_flex.tile([128, 512], fp32, tag="flex")
                for dk in range(2):
                    nc.tensor.transpose(
                        pt[:, dk * 128 : (dk + 1) * 128],
                        Q1[:, dk * 128 : (dk + 1) * 128],
                        ident,
                    )
                for dk in range(2):
                    nc.vector.tensor_scalar_mul(
                        QT3[:, dk, 0:128],
                        pt[:, dk * 128 : (dk + 1) * 128],
                        w_sb[:, 4 + dk : 5 + dk],
                    )
                    nc.vector.tensor_copy(QT3[:, dk, 128:130], w1r2[:, dk])

                # ---- CT (transpose C): 4 transposes per PSUM bank, 1 copy ----
                CT = p_ct.tile([128, 2, LC], mmdt, tag="ct")
                for dk in range(2):
                    for h in range(2):
                        pt = ps_flex.tile([128, 512], fp32, tag="flex")
                        for jj in range(4):
                            j = h * 4 + jj
                            nc.tensor.transpose(
                                pt[:, jj * 128 : (jj + 1) * 128],
                                Cb[:, j, dk * 128 : (dk + 1) * 128],
                                ident,
                            )
                        dst = CT[:, dk, h * 512 : (h + 1) * 512]
                        if (dk * 2 + h) % 2 == 0:
                            nc.scalar.copy(dst, pt)
                        else:
                            nc.vector.tensor_copy(dst, pt)

                # ---- ST = (Q*w3) @ C^T, E_q = exp(ST + bias_q) ----
                E_q = p_e.tile([128, LC], mmdt, tag="eq")
                for h in range(2):
                    st = ps_flex.tile([128, 512], fp32, tag="flex")
                    for dk in range(2):
                        nc.tensor.matmul(
                            st,
                            QT3[:, dk, 0:128],
                            CT[:, dk, h * 512 : (h + 1) * 512],
                            start=(dk == 0),
                            stop=(dk == 1),
                        )
                    nc.scalar.activation(
                        E_q[:, h * 512 : (h + 1) * 512], st, EXP, bias=bias_q
                    )

                # ---- STT tiles: [c, q | rc] ; Xg = exp(STT + rc + cneg) ----
                Xg = p_xg.tile([128, KC, 128], mmdt, tag="xg")
                for j in range(KC):
                    stt = ps_stt.tile([128, 130], fp32, tag="stt")
                    for dk in range(2):
                        nc.tensor.matmul(
                            stt,
                            CT[:, dk, j * 128 : (j + 1) * 128],
                            QT3[:, dk],
                            start=(dk == 0),
                            stop=(dk == 1),
                        )
                    bias_c = p_sm.tile([128, 1], fp32, tag="biasc")
                    nc.vector.tensor_add(bias_c, stt[:, 128:129], cneg[:, j : j + 1])
                    nc.scalar.activation(Xg[:, j], stt[:, 0:128], EXP, bias=bias_c)

                # ---- T1s = (Sc^T C) = (Xg^T @ [C|1]) normalized ----
                t1 = ps_t1.tile([128, D + 2], fp32, tag="t1")
                for j in range(KC):
                    nc.tensor.matmul(
                        t1,
                        Xg[:, j],
                        C1r[:, j],
                        start=(j == 0),
                        stop=(j == KC - 1),
                    )
                recipT = p_sm.tile([128, 1], fp32, tag="recipT")
                nc.vector.reciprocal(recipT, t1[:, D : D + 1])
                T1s = p_sm.tile([128, D], mmdt, tag="t1s")
                nc.vector.tensor_scalar_mul(T1s, t1[:, 0:D], recipT)

                # ---- per c-tile: A / CA / CBm into osb segments ----
                for j in range(KC):
                    eq_j = E_q[:, j * 128 : (j + 1) * 128]
                    psA = ps_a.tile([128, D + 2], fp32, tag="psa")
                    nc.tensor.matmul(psA, eq_j, Q1r[:], start=True, stop=True)
                    psB = ps_b.tile([128, D], fp32, tag="psb")
                    nc.tensor.matmul(psB, eq_j, T1s[:], start=True, stop=True)

                    rr = p_sm.tile([128, 1], fp32, tag="rr")
                    nc.vector.reciprocal(rr, psA[:, D : D + 1])

                    # A = psA * rr  (ACT, per-partition scale)
                    nc.scalar.mul(osb[:, j, D : 2 * D], psA[:, 0:D], rr)
                    # CA = C * A  (GPSIMD, reads the extracted A)
                    nc.gpsimd.tensor_mul(
                        osb[:, j, 2 * D : 3 * D],
                        osb[:, j, 0:D],
                        osb[:, j, D : 2 * D],
                    )
                    # CBm = (psB * rr) * C  (DVE fused)
                    nc.vector.scalar_tensor_tensor(
                        osb[:, j, 3 * D : 4 * D],
                        psB,
                        rr,
                        osb[:, j, 0:D],
                        MULT,
                        MULT,
                    )

                # one 4MB store per batch: 32KB contiguous per partition
                nc.sync.dma_start(
                    out_d.ap()[b].rearrange("(p j) n -> p j n", p=128), osb
                )

    nc.compile()
    return nc


def _get_nc():
    if "nc" not in _CACHE:
        _CACHE["nc"] = _build_nc()
    return _CACHE["nc"]


def _make_in_maps(C, Q, cmask, qmask, Wo_w):
    C = np.ascontiguousarray(C, dtype=np.float32)
    Q = np.ascontiguousarray(Q, dtype=np.float32)
    cneg = ((1.0 - cmask.astype(np.float32)) * NEG_INF).astype(np.float32)
    qneg = ((1.0 - qmask.astype(np.float32)) * NEG_INF).astype(np.float32)
    # c = 8p + j mapping: row-major [128, KC] — plain reshape
    cneg = np.ascontiguousarray(cneg.reshape(B_FULL, 128, KC))
    qneg = np.ascontiguousarray(qneg.reshape(B_FULL, 128, 1))
    Wo_w = Wo_w.astype(np.float32)
    w_pk = np.ascontiguousarray(Wo_w.reshape(6, 128).T)
    w2bc = np.ascontiguousarray(np.broadcast_to(Wo_w[D : 2 * D], (128, D)))
    in_maps = []
    for i in range(N_CORES):
        sl = slice(i * NB, (i + 1) * NB)
        in_maps.append(
            {
                "C": np.ascontiguousarray(C[sl]),
                "Q": np.ascontiguousarray(Q[sl]),
                "cneg": np.ascontiguousarray(cneg[sl]),
                "qneg": np.ascontiguousarray(qneg[sl]),
                "w_pk": w_pk,
                "w2bc": w2bc,
            }
        )
    return in_maps


def kernel(C, Q, cmask, qmask, Wo_w, Wo_b):
    from concourse.bass_utils import run_bass_kernel_spmd

    nc = _get_nc()
    in_maps = _make_in_maps(C, Q, cmask, qmask, Wo_w)
    res = run_bass_kernel_spmd(nc, in_maps, core_ids=list(range(N_CORES)))
    out = np.concatenate([res.results[i]["out"] for i in range(N_CORES)], axis=0)
    return out


# revision 14
# speedup vs baseline: 1.1956x; 1.0410x over previous
"""CQAttention Trainium2 kernel.

Math (per batch b):
  S = (C*w3) @ Q^T + (C@w1)[:,None] + (Q@w2)[None,:] (+bias, dropped: softmax-invariant)
  Sq = softmax over q of qmask-masked S ; Sc = softmax over c of cmask-masked S
  A = Sq@Q ; Bm = Sq @ (Sc^T @ C) ; out = [C | A | C*A | C*Bm]

Device algorithm (no max-subtraction: |S| < 1 so exp is safe; masks become
additive -1e30 terms):
  ST   = (Q*w3) @ C^T                        [q, c]  (PE; stationary QT3)
  E_q  = exp(ST + (rq + qneg)[q])            [q, c]  row-masked (ACT bias)
  STT  = C @ [(Q*w3)^T | w1 w1]              [c, q+2] (PE; stationary CT tile;
         cols 128:130 accumulate rc = C@w1 for free)
  Xg   = exp(STT[:, :128] + (rc + cneg)[c])  [c, q]  col-mask+rc via ACT bias
  T1s  = (Xg^T @ [C|1]) normalized           [q, d]  == Sc^T @ C
  psA  = E_q^T @ [Q|1]                       [c, d+2] unnormalized A | rowsum
  psB  = E_q^T @ T1s                         [c, d]   unnormalized Bm
  A = psA * rr ; CA = C * A ; CBm = C * psB * rr      (rr = 1/rowsum)

Layout: c-row mapping c = 512h + 4p + j (partition p owns 4 consecutive HBM
rows per half-batch h), so each half-batch output store is one DMA with 16KB
contiguous per partition, and C is DMA'd directly into the output tile (no
engine copy, no extra store).

Sharding: data-parallel over batch, 4 batches per core on 8 cores.
"""

import numpy as np

NEG_INF = -1e30
B_FULL, LC, LQ, D = 32, 1024, 128, 256
N_CORES = 8
NB = B_FULL // N_CORES  # batches per core
KC = LC // 128  # c-tiles per batch (8)

_CACHE = {}


def _build_nc():
    import concourse.bacc as bacc
    import concourse.mybir as mybir
    from concourse import tile
    from concourse.masks import make_identity

    fp32 = mybir.dt.float32
    fp32r = mybir.dt.float32r
    mmdt = fp32r
    MULT = mybir.AluOpType.mult
    ADD = mybir.AluOpType.add
    EXP = mybir.ActivationFunctionType.Exp

    nc = bacc.Bacc("TRN2", target_bir_lowering=False, debug=False)

    C_d = nc.dram_tensor("C", [NB, LC, D], fp32, kind="ExternalInput")
    Q_d = nc.dram_tensor("Q", [NB, LQ, D], fp32, kind="ExternalInput")
    cneg_d = nc.dram_tensor("cneg", [NB, 128, KC], fp32, kind="ExternalInput")
    qneg_d = nc.dram_tensor("qneg", [NB, 128, 1], fp32, kind="ExternalInput")
    w_d = nc.dram_tensor("w_pk", [128, 6], fp32, kind="ExternalInput")
    w2bc_d = nc.dram_tensor("w2bc", [128, D], fp32, kind="ExternalInput")
    out_d = nc.dram_tensor("out", [NB, LC, 4 * D], fp32, kind="ExternalOutput")

    with tile.TileContext(nc) as tc:
        with (
            tc.tile_pool(name="const", bufs=1) as const,
            tc.tile_pool(name="crpool", bufs=2) as p_cr,
            tc.tile_pool(name="qpool", bufs=NB) as p_q,
            tc.tile_pool(name="mpool", bufs=NB) as p_m,
            tc.tile_pool(name="ctpool", bufs=3) as p_ct,
            tc.tile_pool(name="qtpool", bufs=2) as p_qt,
            tc.tile_pool(name="epool", bufs=2) as p_e,
            tc.tile_pool(name="xgpool", bufs=2) as p_xg,
            tc.tile_pool(name="opool", bufs=6) as p_o,
            tc.tile_pool(name="smpool", bufs=4) as p_sm,
            tc.tile_pool(name="flex", bufs=3, space="PSUM") as ps_flex,
            tc.tile_pool(name="psstt", bufs=2, space="PSUM") as ps_stt,
            tc.tile_pool(name="pst1", bufs=1, space="PSUM") as ps_t1,
            tc.tile_pool(name="psa", bufs=1, space="PSUM") as ps_a,
            tc.tile_pool(name="psb", bufs=1, space="PSUM") as ps_b,
        ):
            ident = const.tile([128, 128], fp32)
            make_identity(nc, ident)
            w_sb = const.tile([128, 6], fp32)
            nc.sync.dma_start(w_sb, w_d.ap())
            w2bc = const.tile([128, D], fp32)
            nc.sync.dma_start(w2bc, w2bc_d.ap())
            # duplicated-column w1 (fp32r; rides as 2 extra moving cols of STT)
            w1r2 = const.tile([128, 2, 2], mmdt, tag="w1r2")
            for dk in range(2):
                for j in range(2):
                    nc.vector.tensor_copy(w1r2[:, dk, j : j + 1], w_sb[:, dk : dk + 1])
            # warm up the ACT exp table while input DMAs run
            wtmp = const.tile([128, 1], fp32, tag="wtmp")
            nc.scalar.activation(wtmp, w_sb[:, 0:1], EXP)

            # C1r slots: fp32r copies of C for the T1 matmul moving operand.
            # The two ones-pad columns are written once per slot and persist.
            cr_slots = []
            for _ in range(2):
                cr = p_cr.tile([128, KC, D + 2], mmdt, tag="cr")
                nc.vector.memset(cr.bitcast(fp32)[:, :, D : D + 2], 1.0)
                cr_slots.append(cr)

            # ---- hoisted input loads for all batches ----
            osbs, Q1s, cnegs, qnegs = [], [], [], []
            for b in range(NB):
                # output accumulation tiles [C | A | C*A | C*Bm] per half-batch;
                # C lands in cols 0:D straight from HBM (c = 512h+4p+j mapping).
                osb = []
                for h in range(2):
                    o = p_o.tile([128, KC // 2, 4 * D], fp32, tag="osb")
                    nc.sync.dma_start(
                        o[:, :, 0:D],
                        C_d.ap()[b, h * 512 : (h + 1) * 512].rearrange(
                            "(p j) d -> p j d", p=128
                        ),
                    )
                    osb.append(o)
                Q1 = p_q.tile([128, D + 2], fp32, tag="q")
                nc.vector.memset(Q1[:, D : D + 2], 1.0)
                nc.sync.dma_start(Q1[:, 0:D], Q_d.ap()[b])
                cneg = p_m.tile([128, KC], fp32, tag="cneg")
                nc.sync.dma_start(cneg, cneg_d.ap()[b])
                qneg = p_m.tile([128, 1], fp32, tag="qneg")
                nc.sync.dma_start(qneg, qneg_d.ap()[b])
                osbs.append(osb)
                Q1s.append(Q1)
                cnegs.append(cneg)
                qnegs.append(qneg)

            for b in range(NB):
                osb, Q1, cneg, qneg = osbs[b], Q1s[b], cnegs[b], qnegs[b]

                def Cb(t):
                    return osb[t // 4][:, t % 4, 0:D]

                # rounded fp32r copies for matmul moving operands
                C1r = cr_slots[b % 2]
                for h in range(2):
                    nc.vector.tensor_copy(
                        C1r[:, h * 4 : (h + 1) * 4, 0:D], osb[h][:, :, 0:D]
                    )
                Q1r = p_q.tile([128, D + 2], mmdt, tag="qr")
                nc.vector.tensor_copy(Q1r, Q1)

                # ---- bias_q = qneg + sum_d Q*w2 ----
                scr = p_sm.tile([128, D], fp32, tag="ttrs")
                nc.gpsimd.tensor_mul(scr, Q1[:, 0:D], w2bc)
                rq = p_sm.tile([128, 1], fp32, tag="rq")
                nc.vector.tensor_reduce(rq, scr, mybir.AxisListType.X, ADD)
                bias_q = p_sm.tile([128, 1], fp32, tag="biasq")
                nc.vector.tensor_add(bias_q, rq, qneg)

                # ---- QT3aug = [(Q^T)*w3 | w1 w1] per d-chunk ----
                QT3 = p_qt.tile([128, 2, 130], mmdt, tag="qtw3")
                pt = ps_flex.tile([128, 512], fp32, tag="flex")
                for dk in range(2):
                    nc.tensor.transpose(
                        pt[:, dk * 128 : (dk + 1) * 128],
                        Q1[:, dk * 128 : (dk + 1) * 128],
                        ident,
                    )
                for dk in range(2):
                    nc.vector.tensor_scalar_mul(
                        QT3[:, dk, 0:128],
                        pt[:, dk * 128 : (dk + 1) * 128],
                        w_sb[:, 4 + dk : 5 + dk],
                    )
                    nc.vector.tensor_copy(QT3[:, dk, 128:130], w1r2[:, dk])

                # ---- CT (transpose C): 4 transposes per PSUM bank, 1 copy ----
                CT = p_ct.tile([128, 2, LC], mmdt, tag="ct")
                for dk in range(2):
                    for h in range(2):
                        pt = ps_flex.tile([128, 512], fp32, tag="flex")
                        for jj in range(4):
                            j = h * 4 + jj
                            nc.tensor.transpose(
                                pt[:, jj * 128 : (jj + 1) * 128],
                                Cb(j)[:, dk * 128 : (dk + 1) * 128],
                                ident,
                            )
                        dst = CT[:, dk, h * 512 : (h + 1) * 512]
                        if (dk * 2 + h) % 2 == 0:
                            nc.scalar.copy(dst, pt)
                        else:
                            nc.vector.tensor_copy(dst, pt)

                # ---- ST = (Q*w3) @ C^T, E_q = exp(ST + bias_q) ----
                E_q = p_e.tile([128, LC], mmdt, tag="eq")
                for h in range(2):
                    st = ps_flex.tile([128, 512], fp32, tag="flex")
                    for dk in range(2):
                        nc.tensor.matmul(
                            st,
                            QT3[:, dk, 0:128],
                            CT[:, dk, h * 512 : (h + 1) * 512],
                            start=(dk == 0),
                            stop=(dk == 1),
                        )
                    nc.scalar.activation(
                        E_q[:, h * 512 : (h + 1) * 512], st, EXP, bias=bias_q
                    )

                # ---- STT tiles: [c, q | rc] ; Xg = exp(STT + rc + cneg) ----
                Xg = p_xg.tile([128, KC, 128], mmdt, tag="xg")
                for j in range(KC):
                    stt = ps_stt.tile([128, 130], fp32, tag="stt")
                    for dk in range(2):
                        nc.tensor.matmul(
                            stt,
                            CT[:, dk, j * 128 : (j + 1) * 128],
                            QT3[:, dk],
                            start=(dk == 0),
                            stop=(dk == 1),
                        )
                    bias_c = p_sm.tile([128, 1], fp32, tag="biasc")
                    nc.vector.tensor_add(bias_c, stt[:, 128:129], cneg[:, j : j + 1])
                    nc.scalar.activation(Xg[:, j], stt[:, 0:128], EXP, bias=bias_c)

                # ---- T1s = (Sc^T C) = (Xg^T @ [C|1]) normalized ----
                t1 = ps_t1.tile([128, D + 2], fp32, tag="t1")
                for j in range(KC):
                    nc.tensor.matmul(
                        t1,
                        Xg[:, j],
                        C1r[:, j],
                        start=(j == 0),
                        stop=(j == KC - 1),
                    )
                recipT = p_sm.tile([128, 1], fp32, tag="recipT")
                nc.vector.reciprocal(recipT, t1[:, D : D + 1])
                T1s = p_sm.tile([128, D], mmdt, tag="t1s")
                nc.vector.tensor_scalar_mul(T1s, t1[:, 0:D], recipT)

                # ---- per c-tile: A / CA / CBm into osb segments ----
                for t in range(KC):
                    h, j = t // 4, t % 4
                    o = osb[h]
                    eq_j = E_q[:, t * 128 : (t + 1) * 128]
                    psA = ps_a.tile([128, D + 2], fp32, tag="psa")
                    nc.tensor.matmul(psA, eq_j, Q1r[:], start=True, stop=True)
                    psB = ps_b.tile([128, D], fp32, tag="psb")
                    nc.tensor.matmul(psB, eq_j, T1s[:], start=True, stop=True)

                    rr = p_sm.tile([128, 1], fp32, tag="rr")
                    nc.vector.reciprocal(rr, psA[:, D : D + 1])

                    # A = psA * rr  (ACT, per-partition scale)
                    nc.scalar.mul(o[:, j, D : 2 * D], psA[:, 0:D], rr)
                    # CA = C * A  (GPSIMD, reads the extracted A)
                    nc.gpsimd.tensor_mul(
                        o[:, j, 2 * D : 3 * D],
                        o[:, j, 0:D],
                        o[:, j, D : 2 * D],
                    )
                    # CBm = (psB * rr) * C  (DVE fused)
                    nc.vector.scalar_tensor_tensor(
                        o[:, j, 3 * D : 4 * D],
                        psB,
                        rr,
                        o[:, j, 0:D],
                        MULT,
                        MULT,
                    )
                    # store each half as soon as its 4 c-tiles are done:
                    # 2MB DMA, 16KB contiguous per partition
                    if j == 3:
                        nc.sync.dma_start(
                            out_d.ap()[b, h * 512 : (h + 1) * 512].rearrange(
                                "(p j) n -> p j n", p=128
                            ),
                            osb[h],
                        )

    nc.compile()
    return nc


def _get_nc():
    if "nc" not in _CACHE:
        _CACHE["nc"] = _build_nc()
    return _CACHE["nc"]


def _make_in_maps(C, Q, cmask, qmask, Wo_w):
    C = np.ascontiguousarray(C, dtype=np.float32)
    Q = np.ascontiguousarray(Q, dtype=np.float32)
    cneg = ((1.0 - cmask.astype(np.float32)) * NEG_INF).astype(np.float32)
    qneg = ((1.0 - qmask.astype(np.float32)) * NEG_INF).astype(np.float32)
    # c = 512h + 4p + j mapping -> [b, p, (h j)]
    cneg = np.ascontiguousarray(
        cneg.reshape(B_FULL, 2, 128, 4).transpose(0, 2, 1, 3).reshape(B_FULL, 128, KC)
    )
    qneg = np.ascontiguousarray(qneg.reshape(B_FULL, 128, 1))
    Wo_w = Wo_w.astype(np.float32)
    w_pk = np.ascontiguousarray(Wo_w.reshape(6, 128).T)
    w2bc = np.ascontiguousarray(np.broadcast_to(Wo_w[D : 2 * D], (128, D)))
    in_maps = []
    for i in range(N_CORES):
        sl = slice(i * NB, (i + 1) * NB)
        in_maps.append(
            {
                "C": np.ascontiguousarray(C[sl]),
                "Q": np.ascontiguousarray(Q[sl]),
                "cneg": np.ascontiguousarray(cneg[sl]),
                "qneg": np.ascontiguousarray(qneg[sl]),
                "w_pk": w_pk,
                "w2bc": w2bc,
            }
        )
    return in_maps


def kernel(C, Q, cmask, qmask, Wo_w, Wo_b):
    from concourse.bass_utils import run_bass_kernel_spmd

    nc = _get_nc()
    in_maps = _make_in_maps(C, Q, cmask, qmask, Wo_w)
    res = run_bass_kernel_spmd(nc, in_maps, core_ids=list(range(N_CORES)))
    out = np.concatenate([res.results[i]["out"] for i in range(N_CORES)], axis=0)
    return out


# revision 16
# speedup vs baseline: 1.1993x; 1.0031x over previous
"""CQAttention Trainium2 kernel.

Math (per batch b):
  S = (C*w3) @ Q^T + (C@w1)[:,None] + (Q@w2)[None,:] (+bias, dropped: softmax-invariant)
  Sq = softmax over q of qmask-masked S ; Sc = softmax over c of cmask-masked S
  A = Sq@Q ; Bm = Sq @ (Sc^T @ C) ; out = [C | A | C*A | C*Bm]

Device algorithm (no max-subtraction: |S| < 1 so exp is safe; masks become
additive -1e30 terms):
  ST   = (Q*w3) @ C^T                        [q, c]  (PE; stationary QT3)
  E_q  = exp(ST + (rq + qneg)[q])            [q, c]  row-masked (ACT bias)
  STT  = C @ [(Q*w3)^T | w1 w1]              [c, q+2] (PE; stationary CT tile;
         cols 128:130 accumulate rc = C@w1 for free)
  Xg   = exp(STT[:, :128] + (rc + cneg)[c])  [c, q]  col-mask+rc via ACT bias
  T1s  = (Xg^T @ [C|1]) normalized           [q, d]  == Sc^T @ C
  psA  = E_q^T @ [Q|1]                       [c, d+2] unnormalized A | rowsum
  psB  = E_q^T @ T1s                         [c, d]   unnormalized Bm
  A = psA * rr ; CA = C * A ; CBm = C * psB * rr      (rr = 1/rowsum)

Layout: c-row mapping c = 512h + 4p + j (partition p owns 4 consecutive HBM
rows per half-batch h), so each half-batch output store is one DMA with 16KB
contiguous per partition, and C is DMA'd directly into the output tile (no
engine copy, no extra store).

Sharding: data-parallel over batch, 4 batches per core on 8 cores.
"""

import numpy as np

NEG_INF = -1e30
B_FULL, LC, LQ, D = 32, 1024, 128, 256
N_CORES = 8
NB = B_FULL // N_CORES  # batches per core
KC = LC // 128  # c-tiles per batch (8)

_CACHE = {}


def _build_nc():
    import concourse.bacc as bacc
    import concourse.mybir as mybir
    from concourse import tile
    from concourse.masks import make_identity

    fp32 = mybir.dt.float32
    fp32r = mybir.dt.float32r
    mmdt = fp32r
    MULT = mybir.AluOpType.mult
    ADD = mybir.AluOpType.add
    EXP = mybir.ActivationFunctionType.Exp

    nc = bacc.Bacc("TRN2", target_bir_lowering=False, debug=False)

    C_d = nc.dram_tensor("C", [NB, LC, D], fp32, kind="ExternalInput")
    Q_d = nc.dram_tensor("Q", [NB, LQ, D], fp32, kind="ExternalInput")
    cneg_d = nc.dram_tensor("cneg", [NB, 128, KC], fp32, kind="ExternalInput")
    qneg_d = nc.dram_tensor("qneg", [NB, 128, 1], fp32, kind="ExternalInput")
    w_d = nc.dram_tensor("w_pk", [128, 6], fp32, kind="ExternalInput")
    w2bc_d = nc.dram_tensor("w2bc", [128, D], fp32, kind="ExternalInput")
    out_d = nc.dram_tensor("out", [NB, LC, 4 * D], fp32, kind="ExternalOutput")

    with tile.TileContext(nc) as tc:
        with (
            tc.tile_pool(name="const", bufs=1) as const,
            tc.tile_pool(name="crpool", bufs=2) as p_cr,
            tc.tile_pool(name="qpool", bufs=NB) as p_q,
            tc.tile_pool(name="mpool", bufs=NB) as p_m,
            tc.tile_pool(name="ctpool", bufs=3) as p_ct,
            tc.tile_pool(name="qtpool", bufs=2) as p_qt,
            tc.tile_pool(name="epool", bufs=3) as p_e,
            tc.tile_pool(name="xgpool", bufs=3) as p_xg,
            tc.tile_pool(name="opool", bufs=6) as p_o,
            tc.tile_pool(name="smpool", bufs=4) as p_sm,
            tc.tile_pool(name="flex", bufs=3, space="PSUM") as ps_flex,
            tc.tile_pool(name="psstt", bufs=2, space="PSUM") as ps_stt,
            tc.tile_pool(name="pst1", bufs=1, space="PSUM") as ps_t1,
            tc.tile_pool(name="psab", bufs=2, space="PSUM") as ps_ab,
        ):
            ident = const.tile([128, 128], fp32)
            make_identity(nc, ident)
            w_sb = const.tile([128, 6], fp32)
            nc.sync.dma_start(w_sb, w_d.ap())
            w2bc = const.tile([128, D], fp32)
            nc.sync.dma_start(w2bc, w2bc_d.ap())
            # duplicated-column w1 (fp32r; rides as 2 extra moving cols of STT)
            w1r2 = const.tile([128, 2, 2], mmdt, tag="w1r2")
            for dk in range(2):
                for j in range(2):
                    nc.vector.tensor_copy(w1r2[:, dk, j : j + 1], w_sb[:, dk : dk + 1])
            # warm up the ACT exp table while input DMAs run
            wtmp = const.tile([128, 1], fp32, tag="wtmp")
            nc.scalar.activation(wtmp, w_sb[:, 0:1], EXP)

            # C1r slots: fp32r copies of C for the T1 matmul moving operand.
            # The two ones-pad columns are written once per slot and persist.
            cr_slots = []
            for _ in range(2):
                cr = p_cr.tile([128, KC, D + 2], mmdt, tag="cr")
                nc.vector.memset(cr.bitcast(fp32)[:, :, D : D + 2], 1.0)
                cr_slots.append(cr)

            # ---- hoisted input loads for all batches ----
            osbs, Q1s, cnegs, qnegs = [], [], [], []
            for b in range(NB):
                # output accumulation tiles [C | A | C*A | C*Bm] per half-batch;
                # C lands in cols 0:D straight from HBM (c = 512h+4p+j mapping).
                osb = []
                for h in range(2):
                    o = p_o.tile([128, KC // 2, 4 * D], fp32, tag="osb")
                    nc.sync.dma_start(
                        o[:, :, 0:D],
                        C_d.ap()[b, h * 512 : (h + 1) * 512].rearrange(
                            "(p j) d -> p j d", p=128
                        ),
                    )
                    osb.append(o)
                Q1 = p_q.tile([128, D + 2], fp32, tag="q")
                nc.vector.memset(Q1[:, D : D + 2], 1.0)
                nc.sync.dma_start(Q1[:, 0:D], Q_d.ap()[b])
                cneg = p_m.tile([128, KC], fp32, tag="cneg")
                nc.sync.dma_start(cneg, cneg_d.ap()[b])
                qneg = p_m.tile([128, 1], fp32, tag="qneg")
                nc.sync.dma_start(qneg, qneg_d.ap()[b])
                osbs.append(osb)
                Q1s.append(Q1)
                cnegs.append(cneg)
                qnegs.append(qneg)

            for b in range(NB):
                osb, Q1, cneg, qneg = osbs[b], Q1s[b], cnegs[b], qnegs[b]

                def Cb(t):
                    return osb[t // 4][:, t % 4, 0:D]

                # rounded fp32r copies for matmul moving operands
                C1r = cr_slots[b % 2]
                nc.vector.tensor_copy(C1r[:, 0:4, 0:D], osb[0][:, :, 0:D])
                nc.scalar.copy(C1r[:, 4:8, 0:D], osb[1][:, :, 0:D])
                Q1r = p_q.tile([128, D + 2], mmdt, tag="qr")
                nc.vector.tensor_copy(Q1r, Q1)

                # ---- bias_q = qneg + sum_d Q*w2 ----
                scr = p_sm.tile([128, D], fp32, tag="ttrs")
                nc.gpsimd.tensor_mul(scr, Q1[:, 0:D], w2bc)
                rq = p_sm.tile([128, 1], fp32, tag="rq")
                nc.vector.tensor_reduce(rq, scr, mybir.AxisListType.X, ADD)
                bias_q = p_sm.tile([128, 1], fp32, tag="biasq")
                nc.vector.tensor_add(bias_q, rq, qneg)

                # ---- QT3aug = [(Q^T)*w3 | w1 w1] per d-chunk ----
                QT3 = p_qt.tile([128, 2, 130], mmdt, tag="qtw3")
                pt = ps_flex.tile([128, 512], fp32, tag="flex")
                for dk in range(2):
                    nc.tensor.transpose(
                        pt[:, dk * 128 : (dk + 1) * 128],
                        Q1[:, dk * 128 : (dk + 1) * 128],
                        ident,
                    )
                for dk in range(2):
                    nc.vector.tensor_scalar_mul(
                        QT3[:, dk, 0:128],
                        pt[:, dk * 128 : (dk + 1) * 128],
                        w_sb[:, 4 + dk : 5 + dk],
                    )
                    nc.vector.tensor_copy(QT3[:, dk, 128:130], w1r2[:, dk])

                # ---- CT (transpose C): 4 transposes per PSUM bank, 1 copy ----
                CT = p_ct.tile([128, 2, LC], mmdt, tag="ct")
                for dk in range(2):
                    for h in range(2):
                        pt = ps_flex.tile([128, 512], fp32, tag="flex")
                        for jj in range(4):
                            j = h * 4 + jj
                            nc.tensor.transpose(
                                pt[:, jj * 128 : (jj + 1) * 128],
                                Cb(j)[:, dk * 128 : (dk + 1) * 128],
                                ident,
                            )
                        dst = CT[:, dk, h * 512 : (h + 1) * 512]
                        if (dk * 2 + h) % 2 == 0:
                            nc.scalar.copy(dst, pt)
                        else:
                            nc.vector.tensor_copy(dst, pt)

                # ---- ST = (Q*w3) @ C^T, E_q = exp(ST + bias_q) ----
                E_q = p_e.tile([128, LC], mmdt, tag="eq")
                for h in range(2):
                    st = ps_flex.tile([128, 512], fp32, tag="flex")
                    for dk in range(2):
                        nc.tensor.matmul(
                            st,
                            QT3[:, dk, 0:128],
                            CT[:, dk, h * 512 : (h + 1) * 512],
                            start=(dk == 0),
                            stop=(dk == 1),
                        )
                    nc.scalar.activation(
                        E_q[:, h * 512 : (h + 1) * 512], st, EXP, bias=bias_q
                    )

                # ---- STT tiles: [c, q | rc] ; Xg = exp(STT + rc + cneg) ----
                Xg = p_xg.tile([128, KC, 128], mmdt, tag="xg")
                for j in range(KC):
                    stt = ps_stt.tile([128, 130], fp32, tag="stt")
                    for dk in range(2):
                        nc.tensor.matmul(
                            stt,
                            CT[:, dk, j * 128 : (j + 1) * 128],
                            QT3[:, dk],
                            start=(dk == 0),
                            stop=(dk == 1),
                        )
                    bias_c = p_sm.tile([128, 1], fp32, tag="biasc")
                    nc.vector.tensor_add(bias_c, stt[:, 128:129], cneg[:, j : j + 1])
                    nc.scalar.activation(Xg[:, j], stt[:, 0:128], EXP, bias=bias_c)

                # ---- T1s = (Sc^T C) = (Xg^T @ [C|1]) normalized ----
                t1 = ps_t1.tile([128, D + 2], fp32, tag="t1")
                for j in range(KC):
                    nc.tensor.matmul(
                        t1,
                        Xg[:, j],
                        C1r[:, j],
                        start=(j == 0),
                        stop=(j == KC - 1),
                    )
                recipT = p_sm.tile([128, 1], fp32, tag="recipT")
                nc.vector.reciprocal(recipT, t1[:, D : D + 1])
                T1s = p_sm.tile([128, D], mmdt, tag="t1s")
                nc.vector.tensor_scalar_mul(T1s, t1[:, 0:D], recipT)

                # ---- per c-tile: A / CA / CBm into osb segments ----
                for t in range(KC):
                    h, j = t // 4, t % 4
                    o = osb[h]
                    eq_j = E_q[:, t * 128 : (t + 1) * 128]
                    psA = ps_ab.tile([128, D + 2], fp32, tag="ab")
                    nc.tensor.matmul(psA, eq_j, Q1r[:], start=True, stop=True)
                    psBt = ps_ab.tile([128, D + 2], fp32, tag="ab")
                    psB = psBt[:, 0:D]
                    nc.tensor.matmul(psB, eq_j, T1s[:], start=True, stop=True)

                    rr = p_sm.tile([128, 1], fp32, tag="rr")
                    nc.vector.reciprocal(rr, psA[:, D : D + 1])

                    # A = psA * rr  (ACT, per-partition scale)
                    nc.scalar.mul(o[:, j, D : 2 * D], psA[:, 0:D], rr)
                    # CA = C * A  (GPSIMD, reads the extracted A)
                    nc.gpsimd.tensor_mul(
                        o[:, j, 2 * D : 3 * D],
                        o[:, j, 0:D],
                        o[:, j, D : 2 * D],
                    )
                    # CBm = (psB * rr) * C  (DVE fused)
                    nc.vector.scalar_tensor_tensor(
                        o[:, j, 3 * D : 4 * D],
                        psB,
                        rr,
                        o[:, j, 0:D],
                        MULT,
                        MULT,
                    )
                    # store each half as soon as its 4 c-tiles are done:
                    # 2MB DMA, 16KB contiguous per partition
                    if j == 3:
                        nc.sync.dma_start(
                            out_d.ap()[b, h * 512 : (h + 1) * 512].rearrange(
                                "(p j) n -> p j n", p=128
                            ),
                            osb[h],
                        )

    nc.compile()
    return nc


def _get_nc():
    if "nc" not in _CACHE:
        _CACHE["nc"] = _build_nc()
    return _CACHE["nc"]


def _make_in_maps(C, Q, cmask, qmask, Wo_w):
    C = np.ascontiguousarray(C, dtype=np.float32)
    Q = np.ascontiguousarray(Q, dtype=np.float32)
    cneg = ((1.0 - cmask.astype(np.float32)) * NEG_INF).astype(np.float32)
    qneg = ((1.0 - qmask.astype(np.float32)) * NEG_INF).astype(np.float32)
    # c = 512h + 4p + j mapping -> [b, p, (h j)]
    cneg = np.ascontiguousarray(
        cneg.reshape(B_FULL, 2, 128, 4).transpose(0, 2, 1, 3).reshape(B_FULL, 128, KC)
    )
    qneg = np.ascontiguousarray(qneg.reshape(B_FULL, 128, 1))
    Wo_w = Wo_w.astype(np.float32)
    w_pk = np.ascontiguousarray(Wo_w.reshape(6, 128).T)
    w2bc = np.ascontiguousarray(np.broadcast_to(Wo_w[D : 2 * D], (128, D)))
    in_maps = []
    for i in range(N_CORES):
        sl = slice(i * NB, (i + 1) * NB)
        in_maps.append(
            {
                "C": np.ascontiguousarray(C[sl]),
                "Q": np.ascontiguousarray(Q[sl]),
                "cneg": np.ascontiguousarray(cneg[sl]),
                "qneg": np.ascontiguousarray(qneg[sl]),
                "w_pk": w_pk,
                "w2bc": w2bc,
            }
        )
    return in_maps


def kernel(C, Q, cmask, qmask, Wo_w, Wo_b):
    from concourse.bass_utils import run_bass_kernel_spmd

    nc = _get_nc()
    in_maps = _make_in_maps(C, Q, cmask, qmask, Wo_w)
    res = run_bass_kernel_spmd(nc, in_maps, core_ids=list(range(N_CORES)))
    out = np.concatenate([res.results[i]["out"] for i in range(N_CORES)], axis=0)
    return out


# revision 19
# speedup vs baseline: 1.2191x; 1.0165x over previous
"""CQAttention Trainium2 kernel.

Math (per batch b):
  S = (C*w3) @ Q^T + (C@w1)[:,None] + (Q@w2)[None,:] (+bias, dropped: softmax-invariant)
  Sq = softmax over q of qmask-masked S ; Sc = softmax over c of cmask-masked S
  A = Sq@Q ; Bm = Sq @ (Sc^T @ C) ; out = [C | A | C*A | C*Bm]

Device algorithm (no max-subtraction: |S| < 1 so exp is safe; masks become
additive -1e30 terms):
  ST   = (Q*w3) @ C^T                        [q, c]  (PE; stationary QT3)
  E_q  = exp(ST + (rq + qneg)[q])            [q, c]  row-masked (ACT bias)
  STT  = C @ [(Q*w3)^T | w1 w1]              [c, q+2] (PE; stationary CT tile;
         cols 128:130 accumulate rc = C@w1 for free)
  Xg   = exp(STT[:, :128] + (rc + cneg)[c])  [c, q]  col-mask+rc via ACT bias
  T1s  = (Xg^T @ [C|1]) normalized           [q, d]  == Sc^T @ C
  psA  = E_q^T @ [Q|1]                       [c, d+2] unnormalized A | rowsum
  psB  = E_q^T @ T1s                         [c, d]   unnormalized Bm
  A = psA * rr ; CA = C * A ; CBm = C * psB * rr      (rr = 1/rowsum)

Layout: c-row mapping c = 512h + 4p + j (partition p owns 4 consecutive HBM
rows per half-batch h), so each half-batch output store is one DMA with 16KB
contiguous per partition, and C is DMA'd directly into the output tile (no
engine copy, no extra store).

Sharding: data-parallel over batch, 4 batches per core on 8 cores.
"""

import numpy as np

NEG_INF = -1e30
B_FULL, LC, LQ, D = 32, 1024, 128, 256
N_CORES = 8
NB = B_FULL // N_CORES  # batches per core
KC = LC // 128  # c-tiles per batch (8)

_CACHE = {}


def _build_nc():
    import concourse.bacc as bacc
    import concourse.mybir as mybir
    from concourse import tile
    from concourse.masks import make_identity

    fp32 = mybir.dt.float32
    fp32r = mybir.dt.float32r
    mmdt = fp32r
    MULT = mybir.AluOpType.mult
    ADD = mybir.AluOpType.add
    EXP = mybir.ActivationFunctionType.Exp

    nc = bacc.Bacc("TRN2", target_bir_lowering=False, debug=False)

    C_d = nc.dram_tensor("C", [NB, LC, D], fp32, kind="ExternalInput")
    Q_d = nc.dram_tensor("Q", [NB, LQ, D], fp32, kind="ExternalInput")
    cneg_d = nc.dram_tensor("cneg", [NB, 128, KC], fp32, kind="ExternalInput")
    qneg_d = nc.dram_tensor("qneg", [NB, 128, 1], fp32, kind="ExternalInput")
    w_d = nc.dram_tensor("w_pk", [128, 6], fp32, kind="ExternalInput")
    w2bc_d = nc.dram_tensor("w2bc", [128, D], fp32, kind="ExternalInput")
    out_d = nc.dram_tensor("out", [NB, LC, 4 * D], fp32, kind="ExternalOutput")

    with tile.TileContext(nc) as tc:
        with (
            tc.tile_pool(name="const", bufs=1) as const,
            tc.tile_pool(name="crpool", bufs=2) as p_cr,
            tc.tile_pool(name="qpool", bufs=NB) as p_q,
            tc.tile_pool(name="mpool", bufs=NB) as p_m,
            tc.tile_pool(name="ctpool", bufs=3) as p_ct,
            tc.tile_pool(name="qtpool", bufs=2) as p_qt,
            tc.tile_pool(name="epool", bufs=3) as p_e,
            tc.tile_pool(name="xgpool", bufs=3) as p_xg,
            tc.tile_pool(name="opool", bufs=6) as p_o,
            tc.tile_pool(name="smpool", bufs=4) as p_sm,
            tc.tile_pool(name="flex", bufs=3, space="PSUM") as ps_flex,
            tc.tile_pool(name="psstt", bufs=2, space="PSUM") as ps_stt,
            tc.tile_pool(name="pst1", bufs=1, space="PSUM") as ps_t1,
            tc.tile_pool(name="psab", bufs=2, space="PSUM") as ps_ab,
        ):
            ident = const.tile([128, 128], fp32)
            make_identity(nc, ident)
            w_sb = const.tile([128, 6], fp32)
            nc.sync.dma_start(w_sb, w_d.ap())
            w2bc = const.tile([128, D], fp32)
            nc.sync.dma_start(w2bc, w2bc_d.ap())
            # duplicated-column w1 (fp32r; rides as 2 extra moving cols of STT)
            w1r2 = const.tile([128, 2, 2], mmdt, tag="w1r2")
            for dk in range(2):
                for j in range(2):
                    nc.vector.tensor_copy(w1r2[:, dk, j : j + 1], w_sb[:, dk : dk + 1])
            # warm up the ACT exp table while input DMAs run
            wtmp = const.tile([128, 1], fp32, tag="wtmp")
            nc.scalar.activation(wtmp, w_sb[:, 0:1], EXP)

            # C1r slots: fp32r copies of C for the T1 matmul moving operand.
            # The two ones-pad columns are written once per slot and persist.
            cr_slots = []
            for _ in range(2):
                cr = p_cr.tile([128, KC, D + 2], mmdt, tag="cr")
                nc.vector.memset(cr.bitcast(fp32)[:, :, D : D + 2], 1.0)
                cr_slots.append(cr)

            # ---- hoisted input loads for all batches ----
            osbs, Q1s, cnegs, qnegs = [], [], [], []
            for b in range(NB):
                # output accumulation tiles [C | A | C*A | C*Bm] per half-batch;
                # C lands in cols 0:D straight from HBM (c = 512h+4p+j mapping).
                osb = []
                for h in range(2):
                    o = p_o.tile([128, KC // 2, 4 * D], fp32, tag="osb")
                    nc.sync.dma_start(
                        o[:, :, 0:D],
                        C_d.ap()[b, h * 512 : (h + 1) * 512].rearrange(
                            "(p j) d -> p j d", p=128
                        ),
                    )
                    osb.append(o)
                Q1 = p_q.tile([128, D + 2], fp32, tag="q")
                nc.vector.memset(Q1[:, D : D + 2], 1.0)
                nc.sync.dma_start(Q1[:, 0:D], Q_d.ap()[b])
                cneg = p_m.tile([128, KC], fp32, tag="cneg")
                nc.sync.dma_start(cneg, cneg_d.ap()[b])
                qneg = p_m.tile([128, 1], fp32, tag="qneg")
                nc.sync.dma_start(qneg, qneg_d.ap()[b])
                osbs.append(osb)
                Q1s.append(Q1)
                cnegs.append(cneg)
                qnegs.append(qneg)

            for b in range(NB):
                osb, Q1, cneg, qneg = osbs[b], Q1s[b], cnegs[b], qnegs[b]

                def Cb(t):
                    return osb[t // 4][:, t % 4, 0:D]

                # rounded fp32r copies for matmul moving operands
                C1r = cr_slots[b % 2]
                nc.vector.tensor_copy(C1r[:, 0:4, 0:D], osb[0][:, :, 0:D])
                nc.scalar.copy(C1r[:, 4:8, 0:D], osb[1][:, :, 0:D])
                Q1r = p_q.tile([128, D + 2], mmdt, tag="qr")
                nc.vector.tensor_copy(Q1r, Q1)

                # ---- bias_q = qneg + sum_d Q*w2 ----
                scr = p_sm.tile([128, D], fp32, tag="ttrs")
                nc.gpsimd.tensor_mul(scr, Q1[:, 0:D], w2bc)
                rq = p_sm.tile([128, 1], fp32, tag="rq")
                nc.vector.tensor_reduce(rq, scr, mybir.AxisListType.X, ADD)
                bias_q = p_sm.tile([128, 1], fp32, tag="biasq")
                nc.vector.tensor_add(bias_q, rq, qneg)

                # ---- QT3aug = [(Q^T)*w3 | w1 w1] per d-chunk ----
                QT3 = p_qt.tile([128, 2, 130], mmdt, tag="qtw3")
                pt = ps_flex.tile([128, 512], fp32, tag="flex")
                for dk in range(2):
                    nc.tensor.transpose(
                        pt[:, dk * 128 : (dk + 1) * 128],
                        Q1[:, dk * 128 : (dk + 1) * 128],
                        ident,
                    )
                for dk in range(2):
                    nc.vector.tensor_scalar_mul(
                        QT3[:, dk, 0:128],
                        pt[:, dk * 128 : (dk + 1) * 128],
                        w_sb[:, 4 + dk : 5 + dk],
                    )
                    nc.vector.tensor_copy(QT3[:, dk, 128:130], w1r2[:, dk])

                # ---- CT (transpose C): 4 transposes per PSUM bank, 1 copy ----
                CT = p_ct.tile([128, 2, LC], mmdt, tag="ct")
                for dk in range(2):
                    for h in range(2):
                        pt = ps_flex.tile([128, 512], fp32, tag="flex")
                        for jj in range(4):
                            j = h * 4 + jj
                            nc.tensor.transpose(
                                pt[:, jj * 128 : (jj + 1) * 128],
                                Cb(j)[:, dk * 128 : (dk + 1) * 128],
                                ident,
                            )
                        dst = CT[:, dk, h * 512 : (h + 1) * 512]
                        if (dk * 2 + h) % 2 == 0:
                            nc.scalar.copy(dst, pt)
                        else:
                            nc.vector.tensor_copy(dst, pt)

                # ---- ST = (Q*w3) @ C^T, E_q = exp(ST + bias_q) ----
                E_q = p_e.tile([128, LC], mmdt, tag="eq")
                for h in range(2):
                    st = ps_flex.tile([128, 512], fp32, tag="flex")
                    for dk in range(2):
                        nc.tensor.matmul(
                            st,
                            QT3[:, dk, 0:128],
                            CT[:, dk, h * 512 : (h + 1) * 512],
                            start=(dk == 0),
                            stop=(dk == 1),
                        )
                    nc.scalar.activation(
                        E_q[:, h * 512 : (h + 1) * 512], st, EXP, bias=bias_q
                    )

                # ---- STT tiles: [c, q | rc] ; Xg = exp(STT + rc + cneg) ----
                Xg = p_xg.tile([128, KC, 128], mmdt, tag="xg")
                for j in range(KC):
                    stt = ps_stt.tile([128, 130], fp32, tag="stt")
                    for dk in range(2):
                        nc.tensor.matmul(
                            stt,
                            CT[:, dk, j * 128 : (j + 1) * 128],
                            QT3[:, dk],
                            start=(dk == 0),
                            stop=(dk == 1),
                        )
                    bias_c = p_sm.tile([128, 1], fp32, tag="biasc")
                    nc.vector.tensor_add(bias_c, stt[:, 128:129], cneg[:, j : j + 1])
                    nc.scalar.activation(Xg[:, j], stt[:, 0:128], EXP, bias=bias_c)

                # ---- T1s = (Sc^T C) = (Xg^T @ [C|1]) normalized ----
                t1 = ps_t1.tile([128, D + 2], fp32, tag="t1")
                for j in range(KC):
                    nc.tensor.matmul(
                        t1,
                        Xg[:, j],
                        C1r[:, j],
                        start=(j == 0),
                        stop=(j == KC - 1),
                    )
                recipT = p_sm.tile([128, 1], fp32, tag="recipT")
                nc.vector.reciprocal(recipT, t1[:, D : D + 1])
                T1s = p_sm.tile([128, D], mmdt, tag="t1s")
                nc.vector.tensor_scalar_mul(T1s, t1[:, 0:D], recipT)

                # ---- per c-tile: A / CA / CBm into osb segments ----
                for t in range(KC):
                    h, j = t // 4, t % 4
                    o = osb[h]
                    eq_j = E_q[:, t * 128 : (t + 1) * 128]
                    psA = ps_ab.tile([128, D + 2], fp32, tag="ab")
                    nc.tensor.matmul(psA, eq_j, Q1r[:], start=True, stop=True)
                    psBt = ps_ab.tile([128, D + 2], fp32, tag="ab")
                    psB = psBt[:, 0:D]
                    nc.tensor.matmul(psB, eq_j, T1s[:], start=True, stop=True)

                    rr = p_sm.tile([128, 1], fp32, tag="rr")
                    nc.vector.reciprocal(rr, psA[:, D : D + 1])

                    # A = psA * rr  (ACT, per-partition scale)
                    nc.scalar.mul(o[:, j, D : 2 * D], psA[:, 0:D], rr)
                    # CA = C * A  (GPSIMD, reads the extracted A)
                    nc.gpsimd.tensor_mul(
                        o[:, j, 2 * D : 3 * D],
                        o[:, j, 0:D],
                        o[:, j, D : 2 * D],
                    )
                    # CBm = (psB * rr) * C  (DVE fused)
                    nc.vector.scalar_tensor_tensor(
                        o[:, j, 3 * D : 4 * D],
                        psB,
                        rr,
                        o[:, j, 0:D],
                        MULT,
                        MULT,
                    )
                    # store each half as soon as its 4 c-tiles are done:
                    # 2MB DMA, 16KB contiguous per partition
                    if j == 3:
                        nc.sync.dma_start(
                            out_d.ap()[b, h * 512 : (h + 1) * 512].rearrange(
                                "(p j) n -> p j n", p=128
                            ),
                            osb[h],
                        )

    nc.compile()
    return nc


def _get_nc():
    if "nc" not in _CACHE:
        _CACHE["nc"] = _build_nc()
    return _CACHE["nc"]


def _make_in_maps(C, Q, cmask, qmask, Wo_w):
    C = np.ascontiguousarray(C, dtype=np.float32)
    Q = np.ascontiguousarray(Q, dtype=np.float32)
    cneg = ((1.0 - cmask.astype(np.float32)) * NEG_INF).astype(np.float32)
    qneg = ((1.0 - qmask.astype(np.float32)) * NEG_INF).astype(np.float32)
    # c = 512h + 4p + j mapping -> [b, p, (h j)]
    cneg = np.ascontiguousarray(
        cneg.reshape(B_FULL, 2, 128, 4).transpose(0, 2, 1, 3).reshape(B_FULL, 128, KC)
    )
    qneg = np.ascontiguousarray(qneg.reshape(B_FULL, 128, 1))
    Wo_w = Wo_w.astype(np.float32)
    w_pk = np.ascontiguousarray(Wo_w.reshape(6, 128).T)
    w2bc = np.ascontiguousarray(np.broadcast_to(Wo_w[D : 2 * D], (128, D)))
    in_maps = []
    for i in range(N_CORES):
        sl = slice(i * NB, (i + 1) * NB)
        in_maps.append(
            {
                "C": np.ascontiguousarray(C[sl]),
                "Q": np.ascontiguousarray(Q[sl]),
                "cneg": np.ascontiguousarray(cneg[sl]),
                "qneg": np.ascontiguousarray(qneg[sl]),
                "w_pk": w_pk,
                "w2bc": w2bc,
            }
        )
    return in_maps


def kernel(C, Q, cmask, qmask, Wo_w, Wo_b):
    from concourse.bass_utils import run_bass_kernel_spmd

    nc = _get_nc()
    in_maps = _make_in_maps(C, Q, cmask, qmask, Wo_w)
    res = run_bass_kernel_spmd(nc, in_maps, core_ids=list(range(N_CORES)))
    out = np.concatenate([res.results[i]["out"] for i in range(N_CORES)], axis=0)
    return out
